# revision 5
# baseline (speedup 1.0000x reference)
"""Trainium2 Bass kernel for the gated equivariant MLP (gnn_message_passing).

Computation per node (channels-last irreps):
  input  : 256x0e | 128x1e | 64x2e                      (dim 960)
  fctp1  : per-l linear + fan-in rescale (+bias on 0e)  -> 384+288 scalars/gates, 192x1e, 96x2e
  gate   : SiLU on 384 scalars, sigmoid gates on 192x1e + 96x2e
  fctp2  : per-l linear + fan-in rescale (+bias on 0e)  -> 256x0e | 128x1e | 64x2e (dim 960)

Strategy: data-parallel over nodes across 8 cores.  On the host the input is
transposed to channel-major and de-interleaved per irrep component so the
device only ever does dense stride-1 DMAs.  fctp1 runs weight-stationary
(nodes on the moving/free axis), the gate runs on ACT/DVE in channel-major
layout, and fctp2 runs activation-stationary (weights moving) so its output
lands node-major in PSUM and can be stored directly without any transposes.
Matmuls are bf16 (fp32 PSUM accumulation); everything else is fp32.
"""

import numpy as np
import ml_dtypes

import concourse.bass as bass
import concourse.bacc as bacc
import concourse.tile as tile
from concourse import mybir
from concourse.bass_utils import run_bass_kernel_spmd

F32 = mybir.dt.float32
BF16 = mybir.dt.bfloat16

N_CORES = 8
N_TOTAL = 65536
NPC = N_TOTAL // N_CORES  # nodes per core

CT = 512   # compute node tile (moving free dim / PSUM bank)
DT = 1024  # input DMA node tile

# fctp1 scalar-path M-blocks of w1_s columns: (col0, P, func)
#   672 = 384 silu scalars (3x128) | 192 l1 gates (128+64) | 96 l2 gates
SBLKS = [
    (0, 128, "silu"),
    (128, 128, "silu"),
    (256, 128, "silu"),
    (384, 128, "sig"),   # g_l1 part a
    (512, 64, "sig"),    # g_l1 part b
    (576, 96, "sig"),    # g_l2
]


def build_program(npc=NPC, rep=1, num_devices=N_CORES, sim_safe=False):
    """Emit the per-core Tile program.  Returns the compiled Bacc object.

    sim_safe=True replaces the HW Silu LUT (not implemented in CoreSim) with
    an exact sigmoid+multiply pair; use only for simulator validation.
    """
    nc = bacc.Bacc("TRN2", target_bir_lowering=False, debug=False,
                   num_devices=num_devices)

    xt = nc.dram_tensor("xt", [960, npc], F32, kind="ExternalInput").ap()
    w1s_d = nc.dram_tensor("w1s", [256, 672], BF16, kind="ExternalInput").ap()
    b1_d = nc.dram_tensor("b1", [672, 1], F32, kind="ExternalInput").ap()
    w1l1_d = nc.dram_tensor("w1l1", [128, 192], BF16, kind="ExternalInput").ap()
    w1l2_d = nc.dram_tensor("w1l2", [128, 96], BF16, kind="ExternalInput").ap()  # duplicated rows
    w2s_d = nc.dram_tensor("w2s", [384, 256], BF16, kind="ExternalInput").ap()
    b2r_d = nc.dram_tensor("b2r", [128, 256], F32, kind="ExternalInput").ap()
    w2l1_d = nc.dram_tensor("w2l1", [192, 128], BF16, kind="ExternalInput").ap()
    w2l2_d = nc.dram_tensor("w2l2", [96, 64], BF16, kind="ExternalInput").ap()
    out = nc.dram_tensor("out", [npc, 960], F32, kind="ExternalOutput").ap()

    with tile.TileContext(nc) as tc:
        _emit(tc, nc, xt, w1s_d, b1_d, w1l1_d, w1l2_d, w2s_d, b2r_d,
              w2l1_d, w2l2_d, out, npc, rep, sim_safe)

    nc.compile()
    return nc


def _emit(tc, nc, xt, w1s_d, b1_d, w1l1_d, w1l2_d, w2s_d, b2r_d,
          w2l1_d, w2l2_d, out, npc, rep, sim_safe=False):
    import contextlib
    ctx = contextlib.ExitStack()
    with ctx:
        consts = ctx.enter_context(tc.tile_pool(name="consts", bufs=1))
        xin = ctx.enter_context(tc.tile_pool(name="xin", bufs=2))
        mid = ctx.enter_context(tc.tile_pool(name="mid", bufs=2))
        outp = ctx.enter_context(tc.tile_pool(name="outp", bufs=3))
        psum = ctx.enter_context(tc.tile_pool(name="psum", bufs=2, space="PSUM"))

        # ---- constants into SBUF (once) ----
        w1s_t = []
        for kb in range(2):
            t = consts.tile([128, 672], BF16, tag=f"w1s{kb}")
            nc.sync.dma_start(t[:], w1s_d[kb * 128:(kb + 1) * 128, :])
            w1s_t.append(t)
        b1_t = []
        for (c0, P, _fn) in SBLKS:
            t = consts.tile([P, 1], F32, tag=f"b1_{c0}")
            nc.sync.dma_start(t[:], b1_d[c0:c0 + P, :])
            b1_t.append(t)
        w1l1_t = consts.tile([128, 192], BF16, tag="w1l1")
        nc.sync.dma_start(w1l1_t[:], w1l1_d[:, :])
        w1l2_t = consts.tile([128, 96], BF16, tag="w1l2")
        nc.sync.dma_start(w1l2_t[:], w1l2_d[:, :])
        w2s_t = []
        for kb in range(3):
            t = consts.tile([128, 256], BF16, tag=f"w2s{kb}")
            nc.sync.dma_start(t[:], w2s_d[kb * 128:(kb + 1) * 128, :])
            w2s_t.append(t)
        b2r_t = consts.tile([128, 256], F32, tag="b2r")
        nc.sync.dma_start(b2r_t[:], b2r_d[:, :])
        w2l1a_t = consts.tile([128, 128], BF16, tag="w2l1a")
        nc.sync.dma_start(w2l1a_t[:], w2l1_d[0:128, :])
        w2l1b_t = consts.tile([64, 128], BF16, tag="w2l1b")
        nc.sync.dma_start(w2l1b_t[:], w2l1_d[128:192, :])
        w2l2_t = consts.tile([96, 64], BF16, tag="w2l2")
        nc.sync.dma_start(w2l2_t[:], w2l2_d[:, :])

        n_dt = npc // DT
        n_ct_per_dt = DT // CT

        for _r in range(rep):
            for idt in range(n_dt):
                d0 = idt * DT
                # ---- input DMA (fp32 HBM -> bf16 SBUF, cast in SWDGE) ----
                # channel blocks: 2x x0, 3x x1 comps, x2 packed (c0|c1),(c2|c3),(c4)
                xb = []
                for cb in range(7):
                    t = xin.tile([128, DT], BF16, tag=f"xb{cb}")
                    nc.gpsimd.dma_start(t[:], xt[cb * 128:(cb + 1) * 128, d0:d0 + DT])
                    xb.append(t)
                t = xin.tile([64, DT], BF16, tag="xb7")
                nc.gpsimd.dma_start(t[:], xt[896:960, d0:d0 + DT])
                xb.append(t)
                # x2 component i -> (tile, partition slice)
                x2map = [(xb[5], 0), (xb[5], 64), (xb[6], 0), (xb[6], 64), (xb[7], 0)]

                for ict in range(n_ct_per_dt):
                    ns = slice(ict * CT, (ict + 1) * CT)
                    n0 = d0 + ict * CT

                    # ---- fctp1 scalar path + gate nonlinearities ----
                    sc_t = []   # 3x [128, CT] bf16 silu outputs
                    g_t = []    # [128],[64],[96] bf16 sigmoid gates
                    for bi, (c0, P, fn) in enumerate(SBLKS):
                        ps = psum.tile([P, CT], F32, tag="ps_s")
                        for kb in range(2):
                            nc.tensor.matmul(
                                ps[:], w1s_t[kb][:, c0:c0 + P], xb[kb][:, ns],
                                start=(kb == 0), stop=(kb == 1))
                        dst = mid.tile([P, CT], BF16, tag=f"sg{bi}")
                        if fn == "silu" and sim_safe:
                            tmp = mid.tile([P, CT], F32, tag=f"sgt{bi}")
                            nc.scalar.activation(
                                tmp[:], ps[:],
                                mybir.ActivationFunctionType.Sigmoid,
                                bias=b1_t[bi][:])
                            nc.vector.scalar_tensor_tensor(
                                dst[:], ps[:], b1_t[bi][:], tmp[:],
                                op0=mybir.AluOpType.add,
                                op1=mybir.AluOpType.mult)
                        else:
                            func = (mybir.ActivationFunctionType.Silu
                                    if fn == "silu"
                                    else mybir.ActivationFunctionType.Sigmoid)
                            nc.scalar.activation(dst[:], ps[:], func,
                                                 bias=b1_t[bi][:])
                        (sc_t if fn == "silu" else g_t).append(dst)

                    # ---- fctp1 l=1, l=2 paths + gating ----
                    z1a, z1b, z2 = [], [], []
                    for i in range(3):
                        ps = psum.tile([128, CT], F32, tag="ps_y")
                        nc.tensor.matmul(ps[:], w1l1_t[:, 0:128], xb[2 + i][:, ns],
                                         start=True, stop=True)
                        z = mid.tile([128, CT], BF16, tag=f"z1a{i}")
                        nc.vector.tensor_mul(z[:], ps[:], g_t[0][:])
                        z1a.append(z)
                        ps = psum.tile([64, CT], F32, tag="ps_y")
                        nc.tensor.matmul(ps[:], w1l1_t[:, 128:192], xb[2 + i][:, ns],
                                         start=True, stop=True)
                        z = mid.tile([64, CT], BF16, tag=f"z1b{i}")
                        nc.vector.tensor_mul(z[:], ps[:], g_t[1][:])
                        z1b.append(z)
                    for i in range(5):
                        xt2, p0 = x2map[i]
                        ps = psum.tile([96, CT], F32, tag="ps_y")
                        nc.tensor.matmul(ps[:], w1l2_t[p0:p0 + 64, :],
                                         xt2[p0:p0 + 64, ns], start=True, stop=True)
                        z = mid.tile([96, CT], BF16, tag=f"z2{i}")
                        nc.vector.tensor_mul(z[:], ps[:], g_t[2][:])
                        z2.append(z)

                    # ---- fctp2 (activations stationary -> node-major out) ----
                    out_sb = outp.tile([128, 4, 960], F32, tag="out_sb")
                    for j in range(4):
                        js = slice(j * 128, (j + 1) * 128)
                        ps0 = psum.tile([128, 256], F32, tag="ps_o")
                        for kb in range(3):
                            nc.tensor.matmul(ps0[:], sc_t[kb][:, js], w2s_t[kb][:],
                                             start=(kb == 0), stop=(kb == 2))
                        nc.vector.tensor_add(out_sb[:, j, 0:256], ps0[:], b2r_t[:])

                        ps1 = psum.tile([128, 128, 3], F32, tag="ps_o")
                        for i in range(3):
                            nc.tensor.matmul(ps1[:, :, i], z1a[i][:, js], w2l1a_t[:],
                                             start=(i == 0), stop=False)
                            nc.tensor.matmul(ps1[:, :, i], z1b[i][:, js], w2l1b_t[:],
                                             start=False, stop=(i == 2))
                        nc.scalar.activation(out_sb[:, j, 256:640],
                                             ps1.rearrange("p a b -> p (a b)"),
                                             mybir.ActivationFunctionType.Copy)

                        ps2 = psum.tile([128, 64, 5], F32, tag="ps_o")
                        for i in range(5):
                            nc.tensor.matmul(ps2[:, :, i], z2[i][:, js], w2l2_t[:],
                                             start=(i == 0), stop=(i == 4))
                        nc.scalar.activation(out_sb[:, j, 640:960],
                                             ps2.rearrange("p a b -> p (a b)"),
                                             mybir.ActivationFunctionType.Copy)

                    dst = out[n0:n0 + CT, :].rearrange("(j p) c -> p j c", p=128)
                    nc.sync.dma_start(dst, out_sb[:])


# ---------------------------------------------------------------------------
# host-side prep + execution
# ---------------------------------------------------------------------------

def _prep_inputs(node_input, node_attr, w1_s, b1_s, w1_l1, w1_l2, w2_s, b2_s,
                 w2_l1, w2_l2):
    """Return (per-core input maps, attr vector or None)."""
    a = np.asarray(node_attr, dtype=np.float32)[:, 0]
    attr = None if np.all(a == 1.0) else a
    x = np.asarray(node_input, dtype=np.float32)
    if attr is not None:
        x = x * a[:, None]

    bf = ml_dtypes.bfloat16
    w1s = (np.asarray(w1_s) / np.sqrt(256.0)).astype(bf)
    b1 = np.asarray(b1_s, dtype=np.float32).reshape(672, 1)
    w1l1 = (np.asarray(w1_l1) / np.sqrt(128.0)).astype(bf)
    w1l2_ = (np.asarray(w1_l2) / np.sqrt(64.0)).astype(bf)
    w1l2 = np.concatenate([w1l2_, w1l2_], axis=0)  # rows duplicated for both PE halves
    w2s = (np.asarray(w2_s) / np.sqrt(384.0)).astype(bf)
    b2r = np.tile(np.asarray(b2_s, dtype=np.float32).reshape(1, 256), (128, 1))
    w2l1 = (np.asarray(w2_l1) / np.sqrt(192.0)).astype(bf)
    w2l2 = (np.asarray(w2_l2) / np.sqrt(96.0)).astype(bf)

    in_maps = []
    for c in range(N_CORES):
        xs = x[c * NPC:(c + 1) * NPC, :]  # (NPC, 960)
        xtc = np.empty((960, NPC), dtype=np.float32)
        xtc[0:256] = xs[:, 0:256].T
        for i in range(3):
            xtc[256 + 128 * i:256 + 128 * (i + 1)] = xs[:, 256 + i:640:3].T
        for i in range(5):
            xtc[640 + 64 * i:640 + 64 * (i + 1)] = xs[:, 640 + i:960:5].T
        in_maps.append({
            "xt": xtc, "w1s": w1s, "b1": b1, "w1l1": w1l1, "w1l2": w1l2,
            "w2s": w2s, "b2r": b2r, "w2l1": w2l1, "w2l2": w2l2,
        })
    return in_maps, attr


def _postprocess(out_full, attr, b2_s):
    if attr is not None:
        b2 = np.asarray(b2_s, dtype=np.float32)
        out_full[:, :256] = (out_full[:, :256] - b2) * attr[:, None] + b2
        out_full[:, 256:] *= attr[:, None]
    return out_full


_PROGRAM_CACHE = {}


def get_program(npc=NPC, rep=1):
    key = (npc, rep)
    if key not in _PROGRAM_CACHE:
        _PROGRAM_CACHE[key] = build_program(npc=npc, rep=rep)
    return _PROGRAM_CACHE[key]


def kernel(node_input, node_attr, w1_s, b1_s, w1_l1, w1_l2, w2_s, b2_s,
           w2_l1, w2_l2):
    in_maps, attr = _prep_inputs(node_input, node_attr, w1_s, b1_s, w1_l1,
                                 w1_l2, w2_s, b2_s, w2_l1, w2_l2)
    nc = get_program()
    res = run_bass_kernel_spmd(nc, in_maps, list(range(N_CORES)))
    out_full = np.concatenate([res.results[c]["out"] for c in range(N_CORES)],
                              axis=0)
    return _postprocess(out_full, attr, b2_s)
